# revision 11
# baseline (speedup 1.0000x reference)
"""Dense mean-field CRF (2-label Potts, gaussian + bilateral pairwise) on 8
Trainium2 NeuronCores.

Math: the bilateral kernel factorizes as S_spatial (separable, sigma=50) o
B_intensity (gaussian gram on the pixel values). B is numerically rank<=48,
so B ~= P @ P.T (Nystrom over 256 landmark intensities, error ~1e-12) and
each mean-field message becomes 48 separable 96x96 convolutions instead of an
85M-entry dense matrix:

    msg = sum_r P_r o (Sy (x) Sx)(10 P_r o h),   h = 2q - 1 = tanh(logit/2)

In h-space the update is  logit = b + msg + 3*conv_g(h) - 13*h  (the
self-exclusion and rowsum terms collapse into these coefficients), so one
Tanh is the only activation. Signed h keeps f32 partial sums random-walking;
total logit noise ~1e-3 vs a minimum decision margin of ~0.02, so the
trajectory tracks the exact computation and the argmax output is exact.

Distribution: the rank dim is sharded across the 8 cores (6 each) with one
AllGather + local 8-way sum per iteration. Iteration 1 is instead replicated
at full rank on every core, hiding under the first-collective bootstrap
barrier that a dummy collective absorbs concurrently.
"""
import sys
sys.path.insert(0, '/opt/trn_rl_repo')
import numpy as np

H = W = 96
N = H * W
NCORES = 8
KRANK = 48
KLOC = KRANK // NCORES
NITER = 5
EPS = 1e-8

_CACHE = {}
LAST_RESULTS = None


# ------------------------- host precomputation -------------------------

def _nystrom_P(f64, krank=KRANK):
    """Rank-k factor P [N, k] with exp(-(fi-fj)^2/400) ~= P @ P.T"""
    t = np.linspace(f64.min() - 1.0, f64.max() + 1.0, 256)
    Ktt = np.exp(-(t[:, None] - t[None, :]) ** 2 / 400.0)
    Kft = np.exp(-(f64[:, None] - t[None, :]) ** 2 / 400.0)
    lam, V = np.linalg.eigh(Ktt)
    keep = lam > lam.max() * 1e-14
    R = V[:, keep] / np.sqrt(lam[keep])
    Praw = Kft @ R
    mu, Wv = np.linalg.eigh(Praw.T @ Praw)
    idx = np.argsort(mu)[::-1][:krank]
    return Praw @ Wv[:, idx]          # float64 [N, krank]


def _rmajor(P3):
    """[y, x, r] -> [96, r*96 + x] float32"""
    return np.ascontiguousarray(
        np.transpose(P3, (0, 2, 1)).reshape(H, -1), dtype=np.float32)


def _host_constants(image, mask):
    img64 = np.asarray(image, dtype=np.float64).reshape(H, W)
    m = np.asarray(mask).reshape(-1)
    f64 = img64.reshape(-1)

    P = _nystrom_P(f64)
    P3 = P.reshape(H, W, KRANK)
    P310 = 10.0 * P3

    idx = np.arange(96, dtype=np.float64)
    d2 = (idx[:, None] - idx[None, :]) ** 2
    b = np.where(m == 0, np.log(EPS), -np.log(EPS))

    to32 = lambda a: np.ascontiguousarray(a, dtype=np.float32)
    shared = {
        "s1": to32(np.exp(-d2 / 5000.0)),
        "g1": to32(np.exp(-d2 / 18.0)),
        "i96": to32(np.eye(96)),
        "cb": to32(b.reshape(H, W)),
        "h0": to32(np.tanh(b / 2.0).reshape(H, W)),
        "pyf10": _rmajor(P310),
        "pyfraw": _rmajor(P3),
    }
    per_core = []
    for c in range(NCORES):
        rs = slice(c * KLOC, (c + 1) * KLOC)
        per_core.append((_rmajor(P310[:, :, rs]), _rmajor(P3[:, :, rs])))
    return per_core, shared


# ------------------------- device program -------------------------

def _build():
    import concourse.bacc as bacc
    import concourse.mybir as mybir
    import concourse.tile as tile

    F32 = mybir.dt.float32
    AF = mybir.ActivationFunctionType
    ALU = mybir.AluOpType
    KW = KLOC * 96          # 576
    KWF = KRANK * 96        # 4608
    RG = [list(range(NCORES))]

    nc = bacc.Bacc("TRN2", target_bir_lowering=False, debug=False,
                   num_devices=NCORES)

    t_in = {}
    for name, shape in [("py10", [96, KW]), ("pyraw", [96, KW]),
                        ("pyf10", [96, KWF]), ("pyfraw", [96, KWF]),
                        ("s1", [96, 96]), ("g1", [96, 96]), ("i96", [96, 96]),
                        ("cb", [96, 96]), ("h0", [96, 96])]:
        t_in[name] = nc.dram_tensor(name, shape, F32, kind="ExternalInput")
    out_t = nc.dram_tensor("logit_out", [96, 96], F32, kind="ExternalOutput")

    with tile.TileContext(nc) as tc:
        with (
            tc.tile_pool(name="const", bufs=1) as cpool,
            tc.tile_pool(name="work", bufs=2) as wpool,
            tc.tile_pool(name="psT", bufs=2, space="PSUM") as psT,
            tc.tile_pool(name="psB", bufs=1, space="PSUM") as psB,
            tc.tile_pool(name="psG", bufs=2, space="PSUM") as psG,
            tc.tile_pool(name="dram", bufs=2, space="DRAM") as dpool,
        ):
            # dummy collective first: absorbs cross-core start skew + comm
            # bootstrap concurrently with input DMAs and iteration 1.
            dml = dpool.tile([8, 4], F32, tag="dml")
            dmo = dpool.tile([64, 4], F32, tag="dmo")
            nc.gpsimd.collective_compute(
                "AllGather", ALU.bypass, replica_groups=RG,
                ins=[dml[:]], outs=[dmo[:]])

            sb = {}
            for name in t_in:
                sb[name] = cpool.tile(list(t_in[name].shape), F32, tag=name,
                                      name=f"sb_{name}")
                nc.sync.dma_start(sb[name][:], t_in[name][:])
            hy = cpool.tile([96, 96], F32, tag="hy")
            nc.sync.dma_start(hy[:], t_in["h0"][:])

            def bilateral_partial(p10, praw, kcnt, tag):
                """msg partial [y, x] = sum_r praw_r o (S (x) S)(p10_r o h)"""
                msg_acc = None
                for r0 in range(0, kcnt, 8):
                    rn = min(8, kcnt - r0)
                    w0, w1 = r0 * 96, (r0 + rn) * 96
                    wp = wpool.tile([96, 8 * 96], F32, tag=f"wp{tag}")
                    nc.vector.tensor_mul(
                        wp[:, :rn * 96].rearrange("p (r x) -> p r x", r=rn),
                        p10[:, w0:w1].rearrange("p (r x) -> p r x", r=rn),
                        hy[:].unsqueeze(1).broadcast_to([96, rn, 96]))
                    # stage A (data-stationary): out_r = (Sy WP_r)^T  [x, y]
                    pt = psT.tile([96, 8 * 128], F32, tag="pt")
                    for r in range(rn):
                        nc.tensor.matmul(pt[:, r * 128:r * 128 + 96],
                                         wp[:, r * 96:(r + 1) * 96],
                                         sb["s1"][:], start=True, stop=True)
                    ts = wpool.tile([96, 8 * 96], F32, tag=f"ts{tag}")
                    nc.vector.tensor_copy(
                        ts[:, :rn * 96].rearrange("p (r y) -> p r y", r=rn),
                        pt[:].rearrange("p (r z) -> p r z", r=8)[:, :rn, 0:96])
                    # stage B (data-stationary): out_r = (Sx T_r)^T  [y, x]
                    pb = psB.tile([96, 8 * 128], F32, tag="pb")
                    for r in range(rn):
                        nc.tensor.matmul(pb[:, r * 128:r * 128 + 96],
                                         ts[:, r * 96:(r + 1) * 96],
                                         sb["s1"][:], start=True, stop=True)
                    mm = wpool.tile([96, 8 * 96], F32, tag=f"mm{tag}")
                    nc.vector.tensor_mul(
                        mm[:, :rn * 96].rearrange("p (r x) -> p r x", r=rn),
                        pb[:].rearrange("p (r z) -> p r z", r=8)[:, :rn, 0:96],
                        praw[:, w0:w1].rearrange("p (r x) -> p r x", r=rn))
                    part = wpool.tile([96, 96], F32, tag=f"part{tag}")
                    nc.vector.tensor_reduce(
                        part[:],
                        mm[:, :rn * 96].rearrange("p (r x) -> p x r", r=rn),
                        axis=mybir.AxisListType.X, op=ALU.add)
                    if msg_acc is None:
                        msg_acc = part
                    else:
                        acc2 = wpool.tile([96, 96], F32, tag=f"acc{tag}")
                        nc.vector.tensor_add(acc2[:], msg_acc[:], part[:])
                        msg_acc = acc2
                return msg_acc

            for it in range(NITER):
                # gaussian term on h (off critical chain): conv_g [y, x]
                pg0 = psG.tile([96, 96], F32, tag="psg")
                nc.tensor.transpose(pg0[:], hy[:], sb["i96"][:])
                hx = wpool.tile([96, 96], F32, tag="hx")
                nc.vector.tensor_copy(hx[:], pg0[:])
                pg1 = psG.tile([96, 96], F32, tag="psg")
                nc.tensor.matmul(pg1[:], sb["g1"][:], hx[:],
                                 start=True, stop=True)          # [x,y] = G H^T
                ga = wpool.tile([96, 96], F32, tag="ga")
                nc.vector.tensor_copy(ga[:], pg1[:])
                pg2 = psG.tile([96, 96], F32, tag="psg")
                nc.tensor.transpose(pg2[:], ga[:], sb["i96"][:])  # [y,x] = H G
                gb = wpool.tile([96, 96], F32, tag="gb")
                nc.vector.tensor_copy(gb[:], pg2[:])
                pg3 = psG.tile([96, 96], F32, tag="psg")
                nc.tensor.matmul(pg3[:], sb["g1"][:], gb[:],
                                 start=True, stop=True)          # [y,x] = G H G
                # base = Cb + 3*conv_g - 13*h   (off critical chain)
                c3 = wpool.tile([96, 96], F32, tag="c3")
                nc.vector.tensor_scalar_mul(c3[:], pg3[:], 3.0)
                h13 = wpool.tile([96, 96], F32, tag="h13")
                nc.vector.tensor_scalar_mul(h13[:], hy[:], 13.0)
                b1 = wpool.tile([96, 96], F32, tag="b1")
                nc.vector.tensor_sub(b1[:], c3[:], h13[:])
                base = wpool.tile([96, 96], F32, tag="base")
                nc.vector.tensor_add(base[:], b1[:], sb["cb"][:])

                logit = wpool.tile([96, 96], F32, tag="logit")
                if it == 0:
                    # replicated full-rank iteration: no collective needed;
                    # runs concurrently with the comm bootstrap barrier.
                    msgf = bilateral_partial(sb["pyf10"], sb["pyfraw"],
                                             KRANK, "f")
                    nc.vector.tensor_add(logit[:], base[:], msgf[:])
                else:
                    msg = bilateral_partial(sb["py10"], sb["pyraw"],
                                            KLOC, "s")
                    cin = dpool.tile([96, 96], F32, tag="cin")
                    cout = dpool.tile([NCORES * 96, 96], F32, tag="cout")
                    nc.sync.dma_start(cin[:], msg[:])
                    nc.gpsimd.collective_compute(
                        "AllGather", ALU.bypass, replica_groups=RG,
                        ins=[cin[:]], outs=[cout[:]])
                    # gathered partials + base as a 9th block, one reduce
                    gath = wpool.tile([96, (NCORES + 1) * 96], F32, tag="gath")
                    nc.vector.tensor_copy(
                        gath[:, NCORES * 96:(NCORES + 1) * 96], base[:])
                    nc.sync.dma_start(
                        gath[:, :NCORES * 96].rearrange(
                            "p (c y) -> p c y", c=NCORES),
                        cout[:].rearrange("(c p) y -> p c y", c=NCORES))
                    nc.vector.tensor_reduce(
                        logit[:],
                        gath[:].rearrange("p (c y) -> p y c", c=NCORES + 1),
                        axis=mybir.AxisListType.X, op=ALU.add)
                if it == NITER - 1:
                    nc.sync.dma_start(out_t[:], logit[:])
                else:
                    hy2 = cpool.tile([96, 96], F32, tag=f"hy{it}",
                                     name=f"hy{it}")
                    nc.scalar.activation(hy2[:], logit[:], AF.Tanh, scale=0.5)
                    hy = hy2

    nc.compile()
    return nc


def _get_nc():
    if "nc" not in _CACHE:
        _CACHE["nc"] = _build()
    return _CACHE["nc"]


# ------------------------- entry point -------------------------

def kernel(image, mask):
    global LAST_RESULTS
    import os
    from concourse.bass_utils import run_bass_kernel_spmd

    per_core, shared = _host_constants(image, mask)
    nc = _get_nc()
    in_maps = []
    for c in range(NCORES):
        m = dict(shared)
        m["py10"], m["pyraw"] = per_core[c]
        in_maps.append(m)
    trace = bool(int(os.environ.get("KERNEL_TRACE", "0")))
    kw = {}
    if trace and os.environ.get("KERNEL_TRACE_ALL"):
        kw["trace_cores"] = list(range(NCORES))
        kw["stitch_traces"] = True
    res = run_bass_kernel_spmd(nc, in_maps, core_ids=list(range(NCORES)),
                               trace=trace, **kw)
    LAST_RESULTS = res
    logit_yx = res.results[0]["logit_out"]          # [y, x]
    pred = (logit_yx < 0).astype(np.float32).reshape(1, 1, H, W)
    return pred


# revision 13
# speedup vs baseline: 1.0488x; 1.0488x over previous
"""Dense mean-field CRF (2-label Potts, gaussian + bilateral pairwise) on 8
Trainium2 NeuronCores.

Math: the bilateral kernel factorizes as S_spatial (separable, sigma=50) o
B_intensity (gaussian gram on the pixel values). B is numerically rank<=48,
so B ~= P @ P.T (Nystrom over 256 landmark intensities, error ~1e-12) and
each mean-field message becomes 48 separable 96x96 convolutions instead of an
85M-entry dense matrix:

    msg = sum_r P_r o (Sy (x) Sx)(10 P_r o h),   h = 2q - 1 = tanh(logit/2)

In h-space the update is  logit = b + msg + 3*conv_g(h) - 13*h  (the
self-exclusion and rowsum terms collapse into these coefficients), so one
Tanh is the only activation. Signed h keeps f32 partial sums random-walking;
total logit noise ~1e-3 vs a minimum decision margin of ~0.02, so the
trajectory tracks the exact computation and the argmax output is exact.

Distribution: the rank dim is sharded across the 8 cores (6 each) with one
AllGather + local 8-way sum per iteration. Iteration 1 is instead replicated
at full rank on every core, hiding under the first-collective bootstrap
barrier that a dummy collective absorbs concurrently.
"""
import sys
sys.path.insert(0, '/opt/trn_rl_repo')
import numpy as np

H = W = 96
N = H * W
NCORES = 8
KRANK = 48
KLOC = KRANK // NCORES
NITER = 5
EPS = 1e-8

_CACHE = {}
LAST_RESULTS = None


# ------------------------- host precomputation -------------------------

def _nystrom_P(f64, krank=KRANK):
    """Rank-k factor P [N, k] with exp(-(fi-fj)^2/400) ~= P @ P.T"""
    t = np.linspace(f64.min() - 1.0, f64.max() + 1.0, 256)
    Ktt = np.exp(-(t[:, None] - t[None, :]) ** 2 / 400.0)
    Kft = np.exp(-(f64[:, None] - t[None, :]) ** 2 / 400.0)
    lam, V = np.linalg.eigh(Ktt)
    keep = lam > lam.max() * 1e-14
    R = V[:, keep] / np.sqrt(lam[keep])
    Praw = Kft @ R
    mu, Wv = np.linalg.eigh(Praw.T @ Praw)
    idx = np.argsort(mu)[::-1][:krank]
    return Praw @ Wv[:, idx]          # float64 [N, krank]


def _rmajor(P3):
    """[y, x, r] -> [96, r*96 + x] float32"""
    return np.ascontiguousarray(
        np.transpose(P3, (0, 2, 1)).reshape(H, -1), dtype=np.float32)


def _host_constants(image, mask):
    img64 = np.asarray(image, dtype=np.float64).reshape(H, W)
    m = np.asarray(mask).reshape(-1)
    f64 = img64.reshape(-1)

    P = _nystrom_P(f64)
    P3 = P.reshape(H, W, KRANK)
    P310 = 10.0 * P3

    idx = np.arange(96, dtype=np.float64)
    d2 = (idx[:, None] - idx[None, :]) ** 2
    b = np.where(m == 0, np.log(EPS), -np.log(EPS))

    to32 = lambda a: np.ascontiguousarray(a, dtype=np.float32)
    shared = {
        "s1": to32(np.exp(-d2 / 5000.0)),
        "g1": to32(np.exp(-d2 / 18.0)),
        "i96": to32(np.eye(96)),
        "cb": to32(b.reshape(H, W)),
        "h0": to32(np.tanh(b / 2.0).reshape(H, W)),
        "pyf10": _rmajor(P310),
        "pyfraw": _rmajor(P3),
    }
    per_core = []
    for c in range(NCORES):
        rs = slice(c * KLOC, (c + 1) * KLOC)
        per_core.append((_rmajor(P310[:, :, rs]), _rmajor(P3[:, :, rs])))
    return per_core, shared


# ------------------------- device program -------------------------

def _build():
    import concourse.bacc as bacc
    import concourse.mybir as mybir
    import concourse.tile as tile

    F32 = mybir.dt.float32
    AF = mybir.ActivationFunctionType
    ALU = mybir.AluOpType
    KW = KLOC * 96          # 576
    KWF = KRANK * 96        # 4608
    RG = [list(range(NCORES))]

    nc = bacc.Bacc("TRN2", target_bir_lowering=False, debug=False,
                   num_devices=NCORES)

    t_in = {}
    for name, shape in [("py10", [96, KW]), ("pyraw", [96, KW]),
                        ("pyf10", [96, KWF]), ("pyfraw", [96, KWF]),
                        ("s1", [96, 96]), ("g1", [96, 96]), ("i96", [96, 96]),
                        ("cb", [96, 96]), ("h0", [96, 96])]:
        t_in[name] = nc.dram_tensor(name, shape, F32, kind="ExternalInput")
    out_t = nc.dram_tensor("logit_out", [96, 96], F32, kind="ExternalOutput")

    with tile.TileContext(nc) as tc:
        with (
            tc.tile_pool(name="const", bufs=1) as cpool,
            tc.tile_pool(name="work", bufs=2) as wpool,
            tc.tile_pool(name="psT", bufs=2, space="PSUM") as psT,
            tc.tile_pool(name="psB", bufs=1, space="PSUM") as psB,
            tc.tile_pool(name="psG", bufs=2, space="PSUM") as psG,
            tc.tile_pool(name="dram", bufs=2, space="DRAM") as dpool,
        ):
            # dummy collective first: absorbs cross-core start skew + comm
            # bootstrap concurrently with input DMAs and iteration 1.
            dml = dpool.tile([8, 4], F32, tag="dml")
            dmo = dpool.tile([64, 4], F32, tag="dmo")
            nc.gpsimd.collective_compute(
                "AllGather", ALU.bypass, replica_groups=RG,
                ins=[dml[:]], outs=[dmo[:]])

            sb = {}
            for name in t_in:
                sb[name] = cpool.tile(list(t_in[name].shape), F32, tag=name,
                                      name=f"sb_{name}")
                nc.sync.dma_start(sb[name][:], t_in[name][:])
            hy = cpool.tile([96, 96], F32, tag="hy")
            nc.sync.dma_start(hy[:], t_in["h0"][:])

            def bilateral_partial(p10, praw, kcnt, tag):
                """msg partial [y, x] = sum_r praw_r o (S (x) S)(p10_r o h)"""
                msg_acc = None
                for r0 in range(0, kcnt, 8):
                    rn = min(8, kcnt - r0)
                    w0, w1 = r0 * 96, (r0 + rn) * 96
                    wp = wpool.tile([96, 8 * 96], F32, tag=f"wp{tag}")
                    nc.vector.tensor_mul(
                        wp[:, :rn * 96].rearrange("p (r x) -> p r x", r=rn),
                        p10[:, w0:w1].rearrange("p (r x) -> p r x", r=rn),
                        hy[:].unsqueeze(1).broadcast_to([96, rn, 96]))
                    # stage A (data-stationary): out_r = (Sy WP_r)^T  [x, y]
                    pt = psT.tile([96, 8 * 128], F32, tag="pt")
                    for r in range(rn):
                        nc.tensor.matmul(pt[:, r * 128:r * 128 + 96],
                                         wp[:, r * 96:(r + 1) * 96],
                                         sb["s1"][:], start=True, stop=True)
                    ts = wpool.tile([96, 8 * 96], F32, tag=f"ts{tag}")
                    nc.vector.tensor_copy(
                        ts[:, :rn * 96].rearrange("p (r y) -> p r y", r=rn),
                        pt[:].rearrange("p (r z) -> p r z", r=8)[:, :rn, 0:96])
                    # stage B (data-stationary): out_r = (Sx T_r)^T  [y, x]
                    pb = psB.tile([96, 8 * 128], F32, tag="pb")
                    for r in range(rn):
                        nc.tensor.matmul(pb[:, r * 128:r * 128 + 96],
                                         ts[:, r * 96:(r + 1) * 96],
                                         sb["s1"][:], start=True, stop=True)
                    mm = wpool.tile([96, 8 * 96], F32, tag=f"mm{tag}")
                    nc.vector.tensor_mul(
                        mm[:, :rn * 96].rearrange("p (r x) -> p r x", r=rn),
                        pb[:].rearrange("p (r z) -> p r z", r=8)[:, :rn, 0:96],
                        praw[:, w0:w1].rearrange("p (r x) -> p r x", r=rn))
                    part = wpool.tile([96, 96], F32, tag=f"part{tag}")
                    nc.vector.tensor_reduce(
                        part[:],
                        mm[:, :rn * 96].rearrange("p (r x) -> p x r", r=rn),
                        axis=mybir.AxisListType.X, op=ALU.add)
                    if msg_acc is None:
                        msg_acc = part
                    else:
                        acc2 = wpool.tile([96, 96], F32, tag=f"acc{tag}")
                        nc.vector.tensor_add(acc2[:], msg_acc[:], part[:])
                        msg_acc = acc2
                return msg_acc

            for it in range(NITER):
                # gaussian term on h (off critical chain): conv_g [y, x]
                pg0 = psG.tile([96, 96], F32, tag="psg")
                nc.tensor.transpose(pg0[:], hy[:], sb["i96"][:])
                hx = wpool.tile([96, 96], F32, tag="hx")
                nc.vector.tensor_copy(hx[:], pg0[:])
                pg1 = psG.tile([96, 96], F32, tag="psg")
                nc.tensor.matmul(pg1[:], sb["g1"][:], hx[:],
                                 start=True, stop=True)          # [x,y] = G H^T
                ga = wpool.tile([96, 96], F32, tag="ga")
                nc.vector.tensor_copy(ga[:], pg1[:])
                pg2 = psG.tile([96, 96], F32, tag="psg")
                nc.tensor.transpose(pg2[:], ga[:], sb["i96"][:])  # [y,x] = H G
                gb = wpool.tile([96, 96], F32, tag="gb")
                nc.vector.tensor_copy(gb[:], pg2[:])
                pg3 = psG.tile([96, 96], F32, tag="psg")
                nc.tensor.matmul(pg3[:], sb["g1"][:], gb[:],
                                 start=True, stop=True)          # [y,x] = G H G
                # base = Cb + 3*conv_g - 13*h   (off critical chain)
                c3 = wpool.tile([96, 96], F32, tag="c3")
                nc.vector.tensor_scalar_mul(c3[:], pg3[:], 3.0)
                h13 = wpool.tile([96, 96], F32, tag="h13")
                nc.vector.tensor_scalar_mul(h13[:], hy[:], 13.0)
                b1 = wpool.tile([96, 96], F32, tag="b1")
                nc.vector.tensor_sub(b1[:], c3[:], h13[:])
                base = wpool.tile([96, 96], F32, tag="base")
                nc.vector.tensor_add(base[:], b1[:], sb["cb"][:])

                logit = wpool.tile([96, 96], F32, tag="logit")
                if it == 0:
                    # replicated full-rank iteration: no collective needed;
                    # runs concurrently with the comm bootstrap barrier.
                    msgf = bilateral_partial(sb["pyf10"], sb["pyfraw"],
                                             KRANK, "f")
                    nc.vector.tensor_add(logit[:], base[:], msgf[:])
                else:
                    msg = bilateral_partial(sb["py10"], sb["pyraw"],
                                            KLOC, "s")
                    cin = dpool.tile([96, 96], F32, tag="cin")
                    cout = dpool.tile([NCORES * 96, 96], F32, tag="cout")
                    nc.sync.dma_start(cin[:], msg[:])
                    nc.gpsimd.collective_compute(
                        "AllGather", ALU.bypass, replica_groups=RG,
                        ins=[cin[:]], outs=[cout[:]])
                    # gathered partials + base as a 9th block, one reduce
                    gath = wpool.tile([96, (NCORES + 1) * 96], F32, tag="gath")
                    nc.vector.tensor_copy(
                        gath[:, NCORES * 96:(NCORES + 1) * 96], base[:])
                    cview = cout[:].rearrange("(c p) y -> p c y", c=NCORES)
                    gview = gath[:, :NCORES * 96].rearrange(
                        "p (c y) -> p c y", c=NCORES)
                    for c0 in range(0, NCORES, 2):
                        nc.sync.dma_start(gview[:, c0:c0 + 2],
                                          cview[:, c0:c0 + 2])
                    nc.vector.tensor_reduce(
                        logit[:],
                        gath[:].rearrange("p (c y) -> p y c", c=NCORES + 1),
                        axis=mybir.AxisListType.X, op=ALU.add)
                if it == NITER - 1:
                    nc.sync.dma_start(out_t[:], logit[:])
                else:
                    hy2 = cpool.tile([96, 96], F32, tag=f"hy{it}",
                                     name=f"hy{it}")
                    nc.scalar.activation(hy2[:], logit[:], AF.Tanh, scale=0.5)
                    hy = hy2

    nc.compile()
    return nc


def _get_nc():
    if "nc" not in _CACHE:
        _CACHE["nc"] = _build()
    return _CACHE["nc"]


# ------------------------- entry point -------------------------

def kernel(image, mask):
    global LAST_RESULTS
    import os
    from concourse.bass_utils import run_bass_kernel_spmd

    per_core, shared = _host_constants(image, mask)
    nc = _get_nc()
    in_maps = []
    for c in range(NCORES):
        m = dict(shared)
        m["py10"], m["pyraw"] = per_core[c]
        in_maps.append(m)
    trace = bool(int(os.environ.get("KERNEL_TRACE", "0")))
    kw = {}
    if trace and os.environ.get("KERNEL_TRACE_ALL"):
        kw["trace_cores"] = list(range(NCORES))
        kw["stitch_traces"] = True
    try:
        res = run_bass_kernel_spmd(nc, in_maps, core_ids=list(range(NCORES)),
                                   trace=trace, **kw)
    except Exception:
        # one retry for transient device hiccups
        res = run_bass_kernel_spmd(nc, in_maps, core_ids=list(range(NCORES)),
                                   trace=trace, **kw)
    LAST_RESULTS = res
    logit_yx = res.results[0]["logit_out"]          # [y, x]
    pred = (logit_yx < 0).astype(np.float32).reshape(1, 1, H, W)
    return pred
